# revision 16
# baseline (speedup 1.0000x reference)
"""Trainium2 Bass kernel for nn_MlpwithSOMModuleCosSimilarity.

Reference computation per (b,k) pair:
  ctx, ent = context[b,k,0], context[b,k,1]            # [128, 768] each
  scores = cos_sim(ctx, ent)                            # [128, 128]
  argmax over ent axis, gather matched ent_n row
  out[l] = MLP(ctx_n[l]) + MLP(ent_n[argmax[l]]) (+2*b2), MLP = relu(x@w1+b1)@w2

Key restructuring for TRN2:
  - Instead of gathering matched ent_n rows (D-wide gather), compute the MLP
    scalar f for ALL ent rows and gather scalars via a one-hot mask matvec:
      mask[l,m] = (scores[l,m] >= rowmax_l);  f_matched = mask @ f_ent.
  - scores in fp32 (argmax must match the fp32 reference; min top-2 gap on
    the dataset is ~3.4e-7), MLP layer 1 in fp32r (tf32-like, ~1e-4 rel),
    MLP layer 2 in fp32 on the PE via PSUM accumulation.
  - Sharding: 256 (b,k) pairs data-parallel over 8 cores, 32 pairs each.

Self-contained: hardcodes shapes B=4, K=64, L=128, D=768, 8 cores.
"""
import sys

sys.path.insert(0, "/opt/trn_rl_repo")

import numpy as np
from contextlib import ExitStack

import concourse.bacc as bacc
import concourse.tile as tile
from concourse import mybir
from concourse.bass import ts
from concourse.bass_utils import run_bass_kernel_spmd
from concourse.masks import make_identity

F32 = mybir.dt.float32
F32R = mybir.dt.float32r

N_CORES = 8
L = 128          # tokens per slice (partition dim everywhere)
D = 768          # feature dim = 6 chunks of 128
DC = 6           # D // 128
N_PAIRS = 32     # (b,k) pairs per core


def build_core_kernel(n_pairs=N_PAIRS, act_f32r_copy=False, mlp2_f32r=True):
    nc = bacc.Bacc("TRN2", target_bir_lowering=False, debug=False)
    x_d = nc.dram_tensor("x", [n_pairs, 2, L, D], F32, kind="ExternalInput").ap()
    w1_d = nc.dram_tensor("w1", [D, D], F32, kind="ExternalInput").ap()
    b1_d = nc.dram_tensor("b1", [D], F32, kind="ExternalInput").ap()
    w2_d = nc.dram_tensor("w2", [D, 1], F32, kind="ExternalInput").ap()
    b2_d = nc.dram_tensor("b2", [1], F32, kind="ExternalInput").ap()
    out_d = nc.dram_tensor("out", [n_pairs, L], F32, kind="ExternalOutput").ap()

    with tile.TileContext(nc) as tc, ExitStack() as ctx:
        consts = ctx.enter_context(tc.tile_pool(name="consts", bufs=1))
        xp = ctx.enter_context(tc.tile_pool(name="xp", bufs=3))
        xnp = ctx.enter_context(tc.tile_pool(name="xnp", bufs=4))
        sqp = ctx.enter_context(tc.tile_pool(name="sqp", bufs=2))
        smallp = ctx.enter_context(tc.tile_pool(name="smallp", bufs=8))
        xt32p = ctx.enter_context(tc.tile_pool(name="xt32p", bufs=4))
        xtrp = ctx.enter_context(tc.tile_pool(name="xtrp", bufs=3))
        relup = ctx.enter_context(tc.tile_pool(name="relup", bufs=2))
        maskp = ctx.enter_context(tc.tile_pool(name="maskp", bufs=4))
        outp = ctx.enter_context(tc.tile_pool(name="outp", bufs=1))
        psT = ctx.enter_context(tc.tile_pool(name="psT", bufs=2, space="PSUM"))
        psH = ctx.enter_context(tc.tile_pool(name="psH", bufs=2, space="PSUM"))
        psS = ctx.enter_context(tc.tile_pool(name="psS", bufs=2, space="PSUM"))

        # ---- constants ----
        # identity first: the first transposes need it, and GPSIMD executes
        # in order — it must not queue behind the big w1 cast-DMA
        ident = consts.tile([128, 128], F32)
        make_identity(nc, ident[:])

        # w1 as fp32r chunks: [d_sub(part), d_chunk, e_chunk, e_sub]
        w1r = consts.tile([128, DC, DC, 128], F32R)
        nc.gpsimd.dma_start(
            w1r[:], w1_d.rearrange("(di ds) (ej es) -> ds di ej es", ds=128, es=128)
        )
        # w2 chunks: [e_sub(part), e_chunk]
        w2_dt = F32R if mlp2_f32r else F32
        w2_sb = consts.tile([128, DC], w2_dt)
        nc.gpsimd.dma_start(w2_sb[:], w2_d.rearrange("(j s) o -> s (j o)", s=128))
        one_11 = consts.tile([1, 1], F32)
        nc.vector.memset(one_11[:], 1.0)
        # b1 chunks: [e_sub(part), e_chunk]
        b1_sb = consts.tile([128, DC], F32)
        nc.sync.dma_start(b1_sb[:], b1_d.rearrange("(j s) -> s j", s=128))
        # 2*b2 broadcast to [128, 1] via K=1 matmul with a ones column
        ones_col = consts.tile([1, 128], F32)
        nc.vector.memset(ones_col[:], 1.0)
        b2_t = consts.tile([1, 1], F32)
        nc.sync.dma_start(b2_t[:], b2_d.rearrange("(a o) -> a o", a=1))
        ps_b2 = psS.tile([128, 2], F32, tag="ps_small")
        nc.tensor.matmul(ps_b2[:, 0:1], ones_col[:], b2_t[:], start=True, stop=True)
        b2x2 = consts.tile([128, 1], F32)
        nc.scalar.mul(b2x2[:], ps_b2[:, 0:1], 2.0)

        out_acc = outp.tile([128, n_pairs], F32)

        # ---- PE warm-up burst: ~5us of dense matmuls during the startup
        # window (first x DMA + norm chain) so HAM reaches K=8/8 (2.4 GHz)
        # before real work arrives ----
        ps_warm = psS.tile([128, 128], F32, tag="ps_small")
        n_warm = 24
        for i in range(n_warm):
            nc.tensor.matmul(
                ps_warm[:], ident[:], ident[:], start=(i == 0), stop=(i == n_warm - 1)
            )
        warm_sink = consts.tile([128, 1], F32)
        nc.vector.tensor_copy(warm_sink[:], ps_warm[:, 0:1])

        assert n_pairs % 2 == 0
        def load_pair(p):
            x_t = xp.tile([128, 2, D], F32, tag="x")
            nc.sync.dma_start(x_t[:], x_d[p].rearrange("s l d -> l s d"))
            return x_t

        def normalize_pair(x_t):
            """norms + normalize both tensors of a pair -> [xn_ctx, xn_ent]"""
            xns = []
            for t in range(2):
                X = x_t[:, t]
                # scratch output of Square is unused; bf16 halves write cost
                sq_scr = sqp.tile([128, D], mybir.dt.bfloat16, tag="sq")
                sumsq = smallp.tile([128, 1], F32, tag="small")
                nc.scalar.activation(
                    sq_scr[:], X, mybir.ActivationFunctionType.Square,
                    accum_out=sumsq[:],
                )
                norm = smallp.tile([128, 1], F32, tag="small")
                nc.scalar.sqrt(norm[:], sumsq[:])
                inv = smallp.tile([128, 1], F32, tag="small")
                nc.vector.reciprocal(inv[:], norm[:])
                xn = xnp.tile([128, D], F32, tag="xn")
                nc.vector.tensor_scalar_mul(xn[:], X, inv[:])
                xns.append(xn)
            return xns

        def process_pair(q, xns, xtr):
            """transposes, psum copies, scores, argmax mask -> maskT tile"""
            xt32s = []
            for t in range(2):
                xn = xns[t]
                ps_t = psT.tile([128, D], F32, tag="ps_t")
                for i in range(DC):
                    nc.tensor.transpose(
                        ps_t[:, ts(i, 128)], xn[:, ts(i, 128)], ident[:]
                    )
                xt32 = xt32p.tile([128, DC, 128], F32, tag="xt32")
                nc.vector.tensor_copy(
                    xt32[:], ps_t[:].rearrange("p (i l) -> p i l", i=DC)
                )
                # fp32r copy reads the SBUF fp32 copy (not PSUM) so the
                # PSUM transpose slot frees after one consumer; ctx copy on
                # DVE, ent copy on ACT (the group-critical one, less queued)
                dst = xtr[:, :, q * 256 + t * 128: q * 256 + (t + 1) * 128]
                if t == 1 or act_f32r_copy:
                    nc.scalar.copy(dst, xt32[:])
                else:
                    nc.vector.tensor_copy(dst, xt32[:])
                xt32s.append(xt32)

            # scores (fp32): [l, m] = ctx_n @ ent_n^T
            ct, et = xt32s
            ps_s = psS.tile([128, 128], F32, tag="ps_small")
            for i in range(DC):
                nc.tensor.matmul(
                    ps_s[:], ct[:, i], et[:, i], start=(i == 0), stop=(i == DC - 1)
                )
            rowmax = smallp.tile([128, 1], F32, tag="small")
            nc.vector.reduce_max(rowmax[:], ps_s[:], axis=mybir.AxisListType.X)
            mask = maskp.tile([128, 128], F32, tag="mask")
            nc.vector.tensor_scalar(
                mask[:], ps_s[:], rowmax[:], None, op0=mybir.AluOpType.is_ge
            )
            ps_mt = psS.tile([128, 128], F32, tag="ps_small")
            nc.tensor.transpose(ps_mt[:], mask[:], ident[:])
            maskT = maskp.tile([128, 128], F32, tag="maskT")
            nc.scalar.copy(maskT[:], ps_mt[:])
            return maskT

        # software pipeline: normalize chain for pair p+1 is emitted BEFORE
        # pair p's heavy DVE/ACT copies, so the in-order DVE queue produces
        # xn(p+1) early and the PE can fill copy-latency with transposes(p+1)
        xn_next = normalize_pair(load_pair(0))
        for g in range(n_pairs // 2):
            # fp32r transposed-normalized rows for this 2-pair group:
            # rows [q*256 + t*128 + l] for pair q in {0,1}, tensor t in {ctx,ent}
            xtr = xtrp.tile([128, DC, 512], F32R, tag="xtr")
            pair_state = []
            for q in range(2):
                p = 2 * g + q
                xn_cur = xn_next
                if p + 1 < n_pairs:
                    xn_next = normalize_pair(load_pair(p + 1))
                pair_state.append(process_pair(q, xn_cur, xtr))

            # ---- MLP layer 1 (fp32r): hT[e,row] for all 512 rows of group ----
            relu_dt = F32R if mlp2_f32r else F32
            relu_h = relup.tile([128, DC, 512], relu_dt, tag="relu")
            for j in range(DC):
                ps_h = psH.tile([128, 512], F32, tag="ps_h")
                for i in range(DC):
                    nc.tensor.matmul(
                        ps_h[:], w1r[:, i, j], xtr[:, i, :],
                        start=(i == 0), stop=(i == DC - 1),
                    )
                nc.scalar.activation(
                    relu_h[:, j], ps_h[:], mybir.ActivationFunctionType.Relu,
                    bias=b1_sb[:, j: j + 1],
                )

            # ---- MLP layer 2: one row-matmul for all 512 rows ----
            # f_row[0, r] = sum_e relu_hT[e, r] * w2[e]
            ps_frow = psS.tile([1, 512], F32, tag="ps_small")
            for j in range(DC):
                nc.tensor.matmul(
                    ps_frow[:], w2_sb[:, j: j + 1], relu_h[:, j, :],
                    start=(j == 0), stop=(j == DC - 1),
                )
            f_row = smallp.tile([1, 512], F32, tag="frow")
            nc.vector.tensor_copy(f_row[:], ps_frow[:])

            # ---- gather + output, per pair ----
            for q in range(2):
                p = 2 * g + q
                maskT = pair_state[q]
                ps_f = psS.tile([128, 2], F32, tag="ps_small")
                # ent column: [128,1] = f_row[ent rows]^T via K=1 matmul
                nc.tensor.matmul(
                    ps_f[:, 1:2],
                    f_row[0:1, q * 256 + 128: q * 256 + 256],
                    one_11[:], start=True, stop=True,
                )
                f_ent = smallp.tile([128, 1], F32, tag="small")
                nc.vector.tensor_copy(f_ent[:], ps_f[:, 1:2])
                # ctx column, then += mask @ f_ent (gather of matched scalars)
                nc.tensor.matmul(
                    ps_f[:, 0:1],
                    f_row[0:1, q * 256: q * 256 + 128],
                    one_11[:], start=True, stop=False,
                )
                nc.tensor.matmul(
                    ps_f[:, 0:1], maskT[:], f_ent[:], start=False, stop=True
                )
                # out[:, p] = f_total + 2*b2
                nc.vector.tensor_scalar(
                    out_acc[:, p: p + 1], ps_f[:, 0:1], b2x2[:], None,
                    op0=mybir.AluOpType.add,
                )

        # ---- store all outputs in one DMA ----
        nc.sync.dma_start(out_d.rearrange("p l -> l p"), out_acc[:])

    nc.compile()
    return nc


_NC_CACHE = {}
TRACE = False           # test harness hook: profile the run
LAST_RESULT = None      # test harness hook: BassKernelResults of last run


def kernel(context, w1, b1, w2, b2):
    B, K, S, Ln, Dn = context.shape
    assert (S, Ln, Dn) == (2, L, D)
    n_total = B * K
    per_core = n_total // N_CORES

    ctx_flat = np.ascontiguousarray(context.reshape(n_total, 2, L, D), dtype=np.float32)
    key = per_core
    if key not in _NC_CACHE:
        _NC_CACHE[key] = build_core_kernel(per_core)
    nc = _NC_CACHE[key]

    in_maps = []
    for c in range(N_CORES):
        in_maps.append({
            "x": np.ascontiguousarray(ctx_flat[c * per_core:(c + 1) * per_core]),
            "w1": np.asarray(w1, dtype=np.float32),
            "b1": np.asarray(b1, dtype=np.float32),
            "w2": np.asarray(w2, dtype=np.float32),
            "b2": np.asarray(b2, dtype=np.float32),
        })
    res = run_bass_kernel_spmd(nc, in_maps, list(range(N_CORES)), trace=TRACE)
    global LAST_RESULT
    LAST_RESULT = res
    outs = [res.results[c]["out"] for c in range(N_CORES)]
    return np.concatenate(outs, axis=0).reshape(B, K, L).astype(np.float32)


if __name__ == "__main__":
    rng = np.random.default_rng(0)
    B, K = 4, 64
    context = rng.standard_normal((B, K, 2, L, D)).astype(np.float32)
    w1 = (rng.standard_normal((D, D)) / np.sqrt(D)).astype(np.float32)
    b1 = np.zeros(D, np.float32)
    w2 = (rng.standard_normal((D, 1)) / np.sqrt(D)).astype(np.float32)
    b2 = np.zeros(1, np.float32)
    out = kernel(context=context, w1=w1, b1=b1, w2=w2, b2=b2)
    print("kernel out", out.shape, out.dtype)


# revision 22
# speedup vs baseline: 1.0657x; 1.0657x over previous
"""Trainium2 Bass kernel for nn_MlpwithSOMModuleCosSimilarity.

Reference computation per (b,k) pair:
  ctx, ent = context[b,k,0], context[b,k,1]            # [128, 768] each
  scores = cos_sim(ctx, ent)                            # [128, 128]
  argmax over ent axis, gather matched ent_n row
  out[l] = MLP(ctx_n[l]) + MLP(ent_n[argmax[l]]) (+2*b2), MLP = relu(x@w1+b1)@w2

Key restructuring for TRN2:
  - Instead of gathering matched ent_n rows (D-wide gather), compute the MLP
    scalar f for ALL ent rows and gather scalars via a one-hot mask matvec:
      mask[l,m] = (scores[l,m] >= rowmax_l);  f_matched = mask @ f_ent.
  - scores in fp32 (argmax must match the fp32 reference; min top-2 gap on
    the dataset is ~3.4e-7), MLP layer 1 in fp32r (tf32-like, ~1e-4 rel),
    MLP layer 2 in fp32 on the PE via PSUM accumulation.
  - Sharding: 256 (b,k) pairs data-parallel over 8 cores, 32 pairs each.

Self-contained: hardcodes shapes B=4, K=64, L=128, D=768, 8 cores.
"""
import sys

sys.path.insert(0, "/opt/trn_rl_repo")

import numpy as np
from contextlib import ExitStack

import concourse.bacc as bacc
import concourse.tile as tile
from concourse import mybir
from concourse.bass import ts
from concourse.bass_utils import run_bass_kernel_spmd
from concourse.masks import make_identity

F32 = mybir.dt.float32
F32R = mybir.dt.float32r

N_CORES = 8
L = 128          # tokens per slice (partition dim everywhere)
D = 768          # feature dim = 6 chunks of 128
DC = 6           # D // 128
N_PAIRS = 32     # (b,k) pairs per core


def build_core_kernel(n_pairs=N_PAIRS, act_f32r_copy=False, mlp2_f32r=True):
    nc = bacc.Bacc("TRN2", target_bir_lowering=False, debug=False)
    x_d = nc.dram_tensor("x", [n_pairs, 2, L, D], F32, kind="ExternalInput").ap()
    w1_d = nc.dram_tensor("w1", [D, D], F32, kind="ExternalInput").ap()
    b1_d = nc.dram_tensor("b1", [D], F32, kind="ExternalInput").ap()
    w2_d = nc.dram_tensor("w2", [D, 1], F32, kind="ExternalInput").ap()
    b2_d = nc.dram_tensor("b2", [1], F32, kind="ExternalInput").ap()
    out_d = nc.dram_tensor("out", [n_pairs, L], F32, kind="ExternalOutput").ap()

    with tile.TileContext(nc) as tc, ExitStack() as ctx:
        consts = ctx.enter_context(tc.tile_pool(name="consts", bufs=1))
        xp = ctx.enter_context(tc.tile_pool(name="xp", bufs=4))
        xnp = ctx.enter_context(tc.tile_pool(name="xnp", bufs=6))
        sqp = ctx.enter_context(tc.tile_pool(name="sqp", bufs=2))
        smallp = ctx.enter_context(tc.tile_pool(name="smallp", bufs=8))
        xt32p = ctx.enter_context(tc.tile_pool(name="xt32p", bufs=4))
        xtrp = ctx.enter_context(tc.tile_pool(name="xtrp", bufs=3))
        relup = ctx.enter_context(tc.tile_pool(name="relup", bufs=2))
        maskp = ctx.enter_context(tc.tile_pool(name="maskp", bufs=4))
        outp = ctx.enter_context(tc.tile_pool(name="outp", bufs=1))
        psT = ctx.enter_context(tc.tile_pool(name="psT", bufs=2, space="PSUM"))
        psH = ctx.enter_context(tc.tile_pool(name="psH", bufs=2, space="PSUM"))
        psS = ctx.enter_context(tc.tile_pool(name="psS", bufs=2, space="PSUM"))

        # ---- constants ----
        # identity first: the first transposes need it, and GPSIMD executes
        # in order — it must not queue behind the big w1 cast-DMA
        ident = consts.tile([128, 128], F32)
        make_identity(nc, ident[:])

        # w1 as fp32r chunks: [d_sub(part), d_chunk, e_chunk, e_sub]
        w1r = consts.tile([128, DC, DC, 128], F32R)
        nc.gpsimd.dma_start(
            w1r[:], w1_d.rearrange("(di ds) (ej es) -> ds di ej es", ds=128, es=128)
        )
        # w2 chunks: [e_sub(part), e_chunk]
        w2_dt = F32R if mlp2_f32r else F32
        w2_sb = consts.tile([128, DC], w2_dt)
        nc.gpsimd.dma_start(w2_sb[:], w2_d.rearrange("(j s) o -> s (j o)", s=128))
        one_11 = consts.tile([1, 1], F32)
        nc.vector.memset(one_11[:], 1.0)
        # b1 chunks: [e_sub(part), e_chunk]
        b1_sb = consts.tile([128, DC], F32)
        nc.sync.dma_start(b1_sb[:], b1_d.rearrange("(j s) -> s j", s=128))
        # 2*b2 broadcast to [128, 1] via K=1 matmul with a ones column
        ones_col = consts.tile([1, 128], F32)
        nc.vector.memset(ones_col[:], 1.0)
        b2_t = consts.tile([1, 1], F32)
        nc.sync.dma_start(b2_t[:], b2_d.rearrange("(a o) -> a o", a=1))
        ps_b2 = psS.tile([128, 2], F32, tag="ps_small")
        nc.tensor.matmul(ps_b2[:, 0:1], ones_col[:], b2_t[:], start=True, stop=True)
        b2x2 = consts.tile([128, 1], F32)
        nc.scalar.mul(b2x2[:], ps_b2[:, 0:1], 2.0)

        out_acc = outp.tile([128, n_pairs], F32)

        # ---- PE warm-up burst: ~5us of dense matmuls during the startup
        # window (first x DMA + norm chain) so HAM reaches K=8/8 (2.4 GHz)
        # before real work arrives ----
        ps_warm = psS.tile([128, 128], F32, tag="ps_small")
        n_warm = 14
        for i in range(n_warm):
            nc.tensor.matmul(
                ps_warm[:], ident[:], ident[:], start=(i == 0), stop=(i == n_warm - 1)
            )
        warm_sink = consts.tile([128, 1], F32)
        nc.vector.tensor_copy(warm_sink[:], ps_warm[:, 0:1])

        assert n_pairs % 2 == 0
        def load_pair(p):
            x_t = xp.tile([128, 2, D], F32, tag="x")
            nc.sync.dma_start(x_t[:], x_d[p].rearrange("s l d -> l s d"))
            return x_t

        def normalize_pair(x_t):
            """norms + normalize both tensors of a pair -> [xn_ctx, xn_ent]"""
            xns = []
            for t in range(2):
                X = x_t[:, t]
                # scratch output of Square is unused; bf16 halves write cost
                sq_scr = sqp.tile([128, D], mybir.dt.bfloat16, tag="sq")
                sumsq = smallp.tile([128, 1], F32, tag="small")
                nc.scalar.activation(
                    sq_scr[:], X, mybir.ActivationFunctionType.Square,
                    accum_out=sumsq[:],
                )
                norm = smallp.tile([128, 1], F32, tag="small")
                nc.scalar.sqrt(norm[:], sumsq[:])
                inv = smallp.tile([128, 1], F32, tag="small")
                nc.vector.reciprocal(inv[:], norm[:])
                xn = xnp.tile([128, D], F32, tag="xn")
                nc.vector.tensor_scalar_mul(xn[:], X, inv[:])
                xns.append(xn)
            return xns

        def process_pair(q, xns, xtr):
            """transposes, psum copies, scores, argmax mask -> maskT tile"""
            xt32s = []
            for t in range(2):
                xn = xns[t]
                ps_t = psT.tile([128, D], F32, tag="ps_t")
                for i in range(DC):
                    nc.tensor.transpose(
                        ps_t[:, ts(i, 128)], xn[:, ts(i, 128)], ident[:]
                    )
                xt32 = xt32p.tile([128, DC, 128], F32, tag="xt32")
                nc.vector.tensor_copy(
                    xt32[:], ps_t[:].rearrange("p (i l) -> p i l", i=DC)
                )
                # fp32r copy reads the SBUF fp32 copy (not PSUM) so the
                # PSUM transpose slot frees after one consumer; ctx copy on
                # DVE, ent copy on ACT (the group-critical one, less queued)
                dst = xtr[:, :, q * 256 + t * 128: q * 256 + (t + 1) * 128]
                if t == 1 or act_f32r_copy:
                    nc.scalar.copy(dst, xt32[:])
                else:
                    nc.vector.tensor_copy(dst, xt32[:])
                xt32s.append(xt32)

            # scores (fp32): [l, m] = ctx_n @ ent_n^T
            ct, et = xt32s
            ps_s = psS.tile([128, 128], F32, tag="ps_small")
            for i in range(DC):
                nc.tensor.matmul(
                    ps_s[:], ct[:, i], et[:, i], start=(i == 0), stop=(i == DC - 1)
                )
            rowmax = smallp.tile([128, 1], F32, tag="small")
            nc.vector.reduce_max(rowmax[:], ps_s[:], axis=mybir.AxisListType.X)
            mask = maskp.tile([128, 128], F32, tag="mask")
            nc.vector.tensor_scalar(
                mask[:], ps_s[:], rowmax[:], None, op0=mybir.AluOpType.is_ge
            )
            return mask

        # software pipeline: normalize chains run TWO pairs ahead of their
        # consumers so the in-order ACT/DVE queues never put the norm chain
        # on the PE critical path
        xn_q = [normalize_pair(load_pair(0)), normalize_pair(load_pair(1))]
        for g in range(n_pairs // 2):
            # fp32r transposed-normalized rows for this 2-pair group:
            # rows [q*256 + t*128 + l] for pair q in {0,1}, tensor t in {ctx,ent}
            xtr = xtrp.tile([128, DC, 512], F32R, tag="xtr")
            pair_state = []
            for q in range(2):
                p = 2 * g + q
                xn_cur = xn_q.pop(0)
                if p + 2 < n_pairs:
                    xn_q.append(normalize_pair(load_pair(p + 2)))
                pair_state.append(process_pair(q, xn_cur, xtr))

            # ---- MLP layer 1 (fp32r): hT[e,row] for all 512 rows of group ----
            relu_dt = F32R if mlp2_f32r else F32
            relu_h = relup.tile([128, DC, 512], relu_dt, tag="relu")
            for j in range(DC):
                ps_h = psH.tile([128, 512], F32, tag="ps_h")
                for i in range(DC):
                    nc.tensor.matmul(
                        ps_h[:], w1r[:, i, j], xtr[:, i, :],
                        start=(i == 0), stop=(i == DC - 1),
                    )
                nc.scalar.activation(
                    relu_h[:, j], ps_h[:], mybir.ActivationFunctionType.Relu,
                    bias=b1_sb[:, j: j + 1],
                )

            # ---- MLP layer 2: one row-matmul for all 512 rows ----
            # f_row[0, r] = sum_e relu_hT[e, r] * w2[e]
            ps_frow = psS.tile([1, 512], F32, tag="ps_small")
            for j in range(DC):
                nc.tensor.matmul(
                    ps_frow[:], w2_sb[:, j: j + 1], relu_h[:, j, :],
                    start=(j == 0), stop=(j == DC - 1),
                )
            f_row = smallp.tile([1, 512], F32, tag="frow")
            nc.vector.tensor_copy(f_row[:], ps_frow[:])

            # ---- gather + output, per pair ----
            for q in range(2):
                p = 2 * g + q
                mask = pair_state[q]
                # broadcast ent-f values across partitions: [128,128] rows = f_ent
                ps_feb = psS.tile([128, 128], F32, tag="ps_small")
                nc.tensor.matmul(
                    ps_feb[:], ones_col[:],
                    f_row[0:1, q * 256 + 128: q * 256 + 256],
                    start=True, stop=True,
                )
                # f_matched[l] = sum_m mask[l,m] * f_ent[m], then + 2*b2
                ttr_scr = maskp.tile([128, 128], F32, tag="ttr")
                nc.vector.tensor_mul(ttr_scr[:], mask[:], ps_feb[:])
                f_match = smallp.tile([128, 1], F32, tag="small")
                nc.vector.tensor_reduce(
                    f_match[:], ttr_scr[:], axis=mybir.AxisListType.X,
                    op=mybir.AluOpType.add,
                )
                nc.vector.tensor_scalar(
                    f_match[:], f_match[:], b2x2[:], None, op0=mybir.AluOpType.add
                )
                # ctx column via K=1 matmul, then out[:, p] = f_ctx + f_match
                ps_f = psS.tile([128, 1], F32, tag="ps_small")
                nc.tensor.matmul(
                    ps_f[:],
                    f_row[0:1, q * 256: q * 256 + 128],
                    one_11[:], start=True, stop=True,
                )
                nc.vector.tensor_add(out_acc[:, p: p + 1], ps_f[:], f_match[:])

        # ---- store all outputs in one DMA ----
        nc.sync.dma_start(out_d.rearrange("p l -> l p"), out_acc[:])

    nc.compile()
    return nc


_NC_CACHE = {}
TRACE = False           # test harness hook: profile the run
LAST_RESULT = None      # test harness hook: BassKernelResults of last run


def kernel(context, w1, b1, w2, b2):
    B, K, S, Ln, Dn = context.shape
    assert (S, Ln, Dn) == (2, L, D)
    n_total = B * K
    per_core = n_total // N_CORES

    ctx_flat = np.ascontiguousarray(context.reshape(n_total, 2, L, D), dtype=np.float32)
    key = per_core
    if key not in _NC_CACHE:
        _NC_CACHE[key] = build_core_kernel(per_core)
    nc = _NC_CACHE[key]

    in_maps = []
    for c in range(N_CORES):
        in_maps.append({
            "x": np.ascontiguousarray(ctx_flat[c * per_core:(c + 1) * per_core]),
            "w1": np.asarray(w1, dtype=np.float32),
            "b1": np.asarray(b1, dtype=np.float32),
            "w2": np.asarray(w2, dtype=np.float32),
            "b2": np.asarray(b2, dtype=np.float32),
        })
    res = run_bass_kernel_spmd(nc, in_maps, list(range(N_CORES)), trace=TRACE)
    global LAST_RESULT
    LAST_RESULT = res
    outs = [res.results[c]["out"] for c in range(N_CORES)]
    return np.concatenate(outs, axis=0).reshape(B, K, L).astype(np.float32)


if __name__ == "__main__":
    rng = np.random.default_rng(0)
    B, K = 4, 64
    context = rng.standard_normal((B, K, 2, L, D)).astype(np.float32)
    w1 = (rng.standard_normal((D, D)) / np.sqrt(D)).astype(np.float32)
    b1 = np.zeros(D, np.float32)
    w2 = (rng.standard_normal((D, 1)) / np.sqrt(D)).astype(np.float32)
    b2 = np.zeros(1, np.float32)
    out = kernel(context=context, w1=w1, b1=b1, w2=w2, b2=b2)
    print("kernel out", out.shape, out.dtype)


# revision 26
# speedup vs baseline: 1.1309x; 1.0611x over previous
"""Trainium2 Bass kernel for nn_MlpwithSOMModuleCosSimilarity.

Reference computation per (b,k) pair:
  ctx, ent = context[b,k,0], context[b,k,1]            # [128, 768] each
  scores = cos_sim(ctx, ent)                            # [128, 128]
  argmax over ent axis, gather matched ent_n row
  out[l] = MLP(ctx_n[l]) + MLP(ent_n[argmax[l]]) (+2*b2), MLP = relu(x@w1+b1)@w2

Key restructuring for TRN2:
  - Instead of gathering matched ent_n rows (D-wide gather), compute the MLP
    scalar f for ALL ent rows and gather scalars via a one-hot mask matvec:
      mask[l,m] = (scores[l,m] >= rowmax_l);  f_matched = mask @ f_ent.
  - scores in fp32 (argmax must match the fp32 reference; min top-2 gap on
    the dataset is ~3.4e-7), MLP layer 1 in fp32r (tf32-like, ~1e-4 rel),
    MLP layer 2 in fp32 on the PE via PSUM accumulation.
  - Sharding: 256 (b,k) pairs data-parallel over 8 cores, 32 pairs each.

Self-contained: hardcodes shapes B=4, K=64, L=128, D=768, 8 cores.
"""
import sys

sys.path.insert(0, "/opt/trn_rl_repo")

import numpy as np
from contextlib import ExitStack

import concourse.bacc as bacc
import concourse.tile as tile
from concourse import mybir
from concourse.bass import ts
from concourse.bass_utils import run_bass_kernel_spmd
from concourse.masks import make_identity

F32 = mybir.dt.float32
F32R = mybir.dt.float32r

N_CORES = 8
L = 128          # tokens per slice (partition dim everywhere)
D = 768          # feature dim = 6 chunks of 128
DC = 6           # D // 128
N_PAIRS = 32     # (b,k) pairs per core


def build_core_kernel(n_pairs=N_PAIRS, act_f32r_copy=False, mlp2_f32r=True):
    nc = bacc.Bacc("TRN2", target_bir_lowering=False, debug=False)
    x_d = nc.dram_tensor("x", [n_pairs, 2, L, D], F32, kind="ExternalInput").ap()
    w1_d = nc.dram_tensor("w1", [D, D], F32, kind="ExternalInput").ap()
    b1_d = nc.dram_tensor("b1", [D], F32, kind="ExternalInput").ap()
    w2_d = nc.dram_tensor("w2", [D, 1], F32, kind="ExternalInput").ap()
    b2_d = nc.dram_tensor("b2", [1], F32, kind="ExternalInput").ap()
    out_d = nc.dram_tensor("out", [n_pairs, L], F32, kind="ExternalOutput").ap()

    with tile.TileContext(nc) as tc, ExitStack() as ctx:
        consts = ctx.enter_context(tc.tile_pool(name="consts", bufs=1))
        xp = ctx.enter_context(tc.tile_pool(name="xp", bufs=5))
        xnp = ctx.enter_context(tc.tile_pool(name="xnp", bufs=8))
        sqp = ctx.enter_context(tc.tile_pool(name="sqp", bufs=2))
        smallp = ctx.enter_context(tc.tile_pool(name="smallp", bufs=8))
        xt32p = ctx.enter_context(tc.tile_pool(name="xt32p", bufs=4))
        xtrp = ctx.enter_context(tc.tile_pool(name="xtrp", bufs=3))
        relup = ctx.enter_context(tc.tile_pool(name="relup", bufs=2))
        maskp = ctx.enter_context(tc.tile_pool(name="maskp", bufs=4))
        outp = ctx.enter_context(tc.tile_pool(name="outp", bufs=1))
        psT = ctx.enter_context(tc.tile_pool(name="psT", bufs=2, space="PSUM"))
        psH = ctx.enter_context(tc.tile_pool(name="psH", bufs=2, space="PSUM"))
        psS = ctx.enter_context(tc.tile_pool(name="psS", bufs=2, space="PSUM"))

        # ---- startup: identity (first transposes need it; GPSIMD is
        # in-order so it must precede the big w1 cast-DMA), then the first
        # x loads BEFORE any other sync-engine DMA ----
        ident = consts.tile([128, 128], F32)
        make_identity(nc, ident[:])

        def load_pair(p):
            x_t = xp.tile([128, 2, D], F32, tag="x")
            nc.sync.dma_start(x_t[:], x_d[p].rearrange("s l d -> l s d"))
            return x_t

        first_x = [load_pair(p) for p in range(3)]

        # ---- constants ----
        # w1 as fp32r chunks: [d_sub(part), d_chunk, e_chunk, e_sub]
        w1r = consts.tile([128, DC, DC, 128], F32R)
        nc.gpsimd.dma_start(
            w1r[:], w1_d.rearrange("(di ds) (ej es) -> ds di ej es", ds=128, es=128)
        )
        # w2 chunks: [e_sub(part), e_chunk]
        w2_dt = F32R if mlp2_f32r else F32
        w2_sb = consts.tile([128, DC], w2_dt)
        nc.gpsimd.dma_start(w2_sb[:], w2_d.rearrange("(j s) o -> s (j o)", s=128))
        one_11 = consts.tile([1, 1], F32)
        nc.vector.memset(one_11[:], 1.0)
        # b1 chunks: [e_sub(part), e_chunk]
        b1_sb = consts.tile([128, DC], F32)
        nc.sync.dma_start(b1_sb[:], b1_d.rearrange("(j s) -> s j", s=128))
        # 2*b2 broadcast to [128, 1] via K=1 matmul with a ones column
        ones_col = consts.tile([1, 128], F32)
        nc.vector.memset(ones_col[:], 1.0)
        b2_t = consts.tile([1, 1], F32)
        nc.sync.dma_start(b2_t[:], b2_d.rearrange("(a o) -> a o", a=1))

        out_acc = outp.tile([128, n_pairs], F32)

        # ---- PE warm-up burst: dense matmuls during the startup window
        # (first x DMA + norm chain) so HAM reaches K=8/8 (2.4 GHz) before
        # real work arrives ----
        ps_warm = psS.tile([128, 128], F32, tag="ps_small")
        n_warm = 14
        for i in range(n_warm):
            nc.tensor.matmul(
                ps_warm[:], ident[:], ident[:], start=(i == 0), stop=(i == n_warm - 1)
            )
        warm_sink = consts.tile([128, 1], F32)
        nc.vector.tensor_copy(warm_sink[:], ps_warm[:, 0:1])

        ps_b2 = psS.tile([128, 2], F32, tag="ps_small")
        nc.tensor.matmul(ps_b2[:, 0:1], ones_col[:], b2_t[:], start=True, stop=True)
        b2x2 = consts.tile([128, 1], F32)
        nc.scalar.mul(b2x2[:], ps_b2[:, 0:1], 2.0)

        assert n_pairs % 2 == 0
        def normalize_pair(x_t):
            """norms + normalize both tensors of a pair -> [xn_ctx, xn_ent]"""
            xns = []
            for t in range(2):
                X = x_t[:, t]
                # scratch output of Square is unused; bf16 halves write cost
                sq_scr = sqp.tile([128, D], mybir.dt.bfloat16, tag="sq")
                sumsq = smallp.tile([128, 1], F32, tag="small")
                nc.scalar.activation(
                    sq_scr[:], X, mybir.ActivationFunctionType.Square,
                    accum_out=sumsq[:],
                )
                norm = smallp.tile([128, 1], F32, tag="small")
                nc.scalar.sqrt(norm[:], sumsq[:])
                inv = smallp.tile([128, 1], F32, tag="small")
                nc.vector.reciprocal(inv[:], norm[:])
                xn = xnp.tile([128, D], F32, tag="xn")
                nc.vector.tensor_scalar_mul(xn[:], X, inv[:])
                xns.append(xn)
            return xns

        def process_pair(q, xns, xtr):
            """transposes, psum copies, scores, argmax mask -> maskT tile"""
            xt32s = []
            for t in range(2):
                xn = xns[t]
                ps_t = psT.tile([128, D], F32, tag="ps_t")
                for i in range(DC):
                    nc.tensor.transpose(
                        ps_t[:, ts(i, 128)], xn[:, ts(i, 128)], ident[:]
                    )
                xt32 = xt32p.tile([128, DC, 128], F32, tag="xt32")
                nc.vector.tensor_copy(
                    xt32[:], ps_t[:].rearrange("p (i l) -> p i l", i=DC)
                )
                # fp32r copy reads the SBUF fp32 copy (not PSUM) so the
                # PSUM transpose slot frees after one consumer; ctx copy on
                # DVE, ent copy on ACT (the group-critical one, less queued)
                dst = xtr[:, :, q * 256 + t * 128: q * 256 + (t + 1) * 128]
                if t == 1 or act_f32r_copy:
                    nc.scalar.copy(dst, xt32[:])
                else:
                    nc.vector.tensor_copy(dst, xt32[:])
                xt32s.append(xt32)

            # scores (fp32): [l, m] = ctx_n @ ent_n^T
            ct, et = xt32s
            ps_s = psS.tile([128, 128], F32, tag="ps_small")
            for i in range(DC):
                nc.tensor.matmul(
                    ps_s[:], ct[:, i], et[:, i], start=(i == 0), stop=(i == DC - 1)
                )
            rowmax = smallp.tile([128, 1], F32, tag="small")
            nc.vector.reduce_max(rowmax[:], ps_s[:], axis=mybir.AxisListType.X)
            mask = maskp.tile([128, 128], F32, tag="mask")
            nc.vector.tensor_scalar(
                mask[:], ps_s[:], rowmax[:], None, op0=mybir.AluOpType.is_ge
            )
            return mask

        # software pipeline: normalize chains run THREE pairs ahead of their
        # consumers so the in-order ACT/DVE queues never put the norm chain
        # on the PE critical path
        xn_q = [normalize_pair(x) for x in first_x]
        for g in range(n_pairs // 2):
            # fp32r transposed-normalized rows for this 2-pair group:
            # rows [q*256 + t*128 + l] for pair q in {0,1}, tensor t in {ctx,ent}
            xtr = xtrp.tile([128, DC, 512], F32R, tag="xtr")
            pair_state = []
            for q in range(2):
                p = 2 * g + q
                xn_cur = xn_q.pop(0)
                if p + 3 < n_pairs:
                    xn_q.append(normalize_pair(load_pair(p + 3)))
                pair_state.append(process_pair(q, xn_cur, xtr))

            # ---- MLP layer 1 (fp32r): hT[e,row] for all 512 rows of group ----
            relu_dt = F32R if mlp2_f32r else F32
            relu_h = relup.tile([128, DC, 512], relu_dt, tag="relu")
            for j in range(DC):
                ps_h = psH.tile([128, 512], F32, tag="ps_h")
                for i in range(DC):
                    nc.tensor.matmul(
                        ps_h[:], w1r[:, i, j], xtr[:, i, :],
                        start=(i == 0), stop=(i == DC - 1),
                    )
                nc.scalar.activation(
                    relu_h[:, j], ps_h[:], mybir.ActivationFunctionType.Relu,
                    bias=b1_sb[:, j: j + 1],
                )

            # ---- MLP layer 2: one row-matmul for all 512 rows ----
            # f_row[0, r] = sum_e relu_hT[e, r] * w2[e]
            ps_frow = psS.tile([1, 512], F32, tag="ps_small")
            for j in range(DC):
                nc.tensor.matmul(
                    ps_frow[:], w2_sb[:, j: j + 1], relu_h[:, j, :],
                    start=(j == 0), stop=(j == DC - 1),
                )
            f_row = smallp.tile([1, 512], F32, tag="frow")
            nc.vector.tensor_copy(f_row[:], ps_frow[:])

            # ---- gather + output, per pair ----
            for q in range(2):
                p = 2 * g + q
                mask = pair_state[q]
                # broadcast ent-f values across partitions: [128,128] rows = f_ent
                ps_feb = psS.tile([128, 128], F32, tag="ps_small")
                nc.tensor.matmul(
                    ps_feb[:], ones_col[:],
                    f_row[0:1, q * 256 + 128: q * 256 + 256],
                    start=True, stop=True,
                )
                # f_matched[l] = sum_m mask[l,m] * f_ent[m], then + 2*b2
                ttr_scr = maskp.tile([128, 128], F32, tag="ttr")
                nc.vector.tensor_mul(ttr_scr[:], mask[:], ps_feb[:])
                f_match = smallp.tile([128, 1], F32, tag="small")
                nc.vector.tensor_reduce(
                    f_match[:], ttr_scr[:], axis=mybir.AxisListType.X,
                    op=mybir.AluOpType.add,
                )
                nc.vector.tensor_scalar(
                    f_match[:], f_match[:], b2x2[:], None, op0=mybir.AluOpType.add
                )
                # ctx column via K=1 matmul, then out[:, p] = f_ctx + f_match
                ps_f = psS.tile([128, 1], F32, tag="ps_small")
                nc.tensor.matmul(
                    ps_f[:],
                    f_row[0:1, q * 256: q * 256 + 128],
                    one_11[:], start=True, stop=True,
                )
                nc.vector.tensor_add(out_acc[:, p: p + 1], ps_f[:], f_match[:])

        # ---- store all outputs in one DMA ----
        nc.sync.dma_start(out_d.rearrange("p l -> l p"), out_acc[:])

    nc.compile()
    return nc


_NC_CACHE = {}
TRACE = False           # test harness hook: profile the run
LAST_RESULT = None      # test harness hook: BassKernelResults of last run


def kernel(context, w1, b1, w2, b2):
    B, K, S, Ln, Dn = context.shape
    assert (S, Ln, Dn) == (2, L, D)
    n_total = B * K
    per_core = n_total // N_CORES

    ctx_flat = np.ascontiguousarray(context.reshape(n_total, 2, L, D), dtype=np.float32)
    key = per_core
    if key not in _NC_CACHE:
        _NC_CACHE[key] = build_core_kernel(per_core)
    nc = _NC_CACHE[key]

    in_maps = []
    for c in range(N_CORES):
        in_maps.append({
            "x": np.ascontiguousarray(ctx_flat[c * per_core:(c + 1) * per_core]),
            "w1": np.asarray(w1, dtype=np.float32),
            "b1": np.asarray(b1, dtype=np.float32),
            "w2": np.asarray(w2, dtype=np.float32),
            "b2": np.asarray(b2, dtype=np.float32),
        })
    res = run_bass_kernel_spmd(nc, in_maps, list(range(N_CORES)), trace=TRACE)
    global LAST_RESULT
    LAST_RESULT = res
    outs = [res.results[c]["out"] for c in range(N_CORES)]
    return np.concatenate(outs, axis=0).reshape(B, K, L).astype(np.float32)


if __name__ == "__main__":
    rng = np.random.default_rng(0)
    B, K = 4, 64
    context = rng.standard_normal((B, K, 2, L, D)).astype(np.float32)
    w1 = (rng.standard_normal((D, D)) / np.sqrt(D)).astype(np.float32)
    b1 = np.zeros(D, np.float32)
    w2 = (rng.standard_normal((D, 1)) / np.sqrt(D)).astype(np.float32)
    b2 = np.zeros(1, np.float32)
    out = kernel(context=context, w1=w1, b1=b1, w2=w2, b2=b2)
    print("kernel out", out.shape, out.dtype)
